# revision 40
# baseline (speedup 1.0000x reference)
"""Trainium2 Bass kernel for nn_Attention_45999099740384.

GQA attention over 8 independent packed sequences (block-diagonal mask with
equal blocks). Sharding: data-parallel over the 8 blocks - one block of
S=1024 tokens per NeuronCore, weights replicated, zero collectives.

Per-core pipeline (bf16 TensorEngine, fp32 PSUM):
  1. v projection in natural layout (xt stationary, wv moving).
  2. q/k projections TRANSPOSED (weight chunks stationary, xt moving) so
     q^T/k^T come out in [head_dim, t] layout directly - no PE transposes.
  3. RoPE applied in transposed layout on VectorE: host-permuted weight
     columns put rotation pairs into (even|odd) partition halves; the
     cross-partition half-swap is done with two 64-partition copies, then
     two mults against duplicated cos / sign-flipped sin tables and an add.
  4. scores computed transposed: ST[s,q] = kT.T @ qT -> ScalarE exp ->
     P^T tiles in SBUF; P@V needs no transpose of P.
  5. softmax row-sums via a ones[128,128] stationary matmul; reciprocal on
     VectorE (keeps ScalarE exp-only: a single activation table, no
     ACT_TABLE_LOAD churn); normalization deferred to after P@V.
  6. wo matmul from the transposed attention output.

Scheduling: the q projection of group g+1 is woven between the score
matmuls of group g's attention units so the PE never waits for ScalarE
exp; for the last group the leading wo matmul chains are woven in instead.
Attention outputs of groups 0-6 bounce through DRAM (SBUF cannot hold
ot_all while xt is still resident); group 7 writes ot_all directly.
"""

import numpy as np
import ml_dtypes

import concourse.bass as bass
import concourse.mybir as mybir
import concourse.tile as tile
from concourse import bacc
from concourse.bass_utils import run_bass_kernel_spmd

# problem constants (hardcoded per task instructions)
DIM = 4096
N_HEADS = 32
HEAD_DIM = 128
N_KV = 8
REP = 4
B = 8
S = 1024
T = B * S

P = 128                  # SBUF partitions
KC = DIM // P            # 32 contraction chunks of 128
KH = KC // 2             # 16 (w_layout half-chunk count)
TT = S // P              # 8 token tiles of 128
TC = S // 512            # 2 token chunks of 512
NCH = 512                # matmul moving free dim
SCALE = HEAD_DIM ** -0.5

F32 = mybir.dt.float32
BF16 = mybir.dt.bfloat16

_CACHE = {}


def build_nc():
    nc = bacc.Bacc("TRN2", target_bir_lowering=False, debug=False, num_devices=8)

    # xt slice-major: [t-slice, k-part, kc, t-within-slice] so slice DMAs are
    # contiguous 8KB/partition (strided layout produced 256B DMA packets)
    xt_d = nc.dram_tensor("xt", [TT, P, KC, P], BF16, kind="ExternalInput")
    c2_d = nc.dram_tensor("c2", [P, S], BF16, kind="ExternalInput")
    s2n_d = nc.dram_tensor("s2n", [P, S], BF16, kind="ExternalInput")
    # wq: [g, h, k-part, kc, d] lhsT chunks (rope-permuted d columns)
    wq_d = nc.dram_tensor("wq", [N_KV, REP, P, KC, HEAD_DIM], BF16,
                          kind="ExternalInput")
    wk_d = nc.dram_tensor("wk", [N_KV, P, KC, HEAD_DIM], BF16,
                          kind="ExternalInput")
    # wv/wo: moving-operand slabs [chunk, half, 128, 16, 512]
    wv_d = nc.dram_tensor("wv", [2, 2, P, KH, NCH], BF16, kind="ExternalInput")
    # wo: [ct, d, h, c] lhsT slabs (stationary, streamed per 128-col tile)
    wo_d = nc.dram_tensor("wo", [KC, P, N_HEADS, P], BF16, kind="ExternalInput")
    # transposed output [DIM, S]; host untransposes
    out_d = nc.dram_tensor("out", [DIM, S], F32, kind="ExternalOutput")
    # attention-output bounce for groups 0..6
    otb_d = nc.dram_tensor("otb", [2, P, 28, NCH], BF16)

    with tile.TileContext(nc) as tc:
        with (
            tc.tile_pool(name="const", bufs=1) as const,
            tc.tile_pool(name="wpool", bufs=1) as wpool,
            tc.tile_pool(name="qtg", bufs=2) as qtg_pool,
            tc.tile_pool(name="ptp", bufs=2) as pt_pool,
            tc.tile_pool(name="scr", bufs=3) as scr_pool,
            tc.tile_pool(name="rcbp", bufs=1) as rcb_pool,
            tc.tile_pool(name="otp", bufs=1) as ot_pool,
            tc.tile_pool(name="outp", bufs=2) as out_pool,
            tc.tile_pool(name="pt8p", bufs=1) as pt8_pool,
            tc.tile_pool(name="psmm", bufs=2, space="PSUM") as ps_pool,
            tc.tile_pool(name="psst", bufs=3, space="PSUM") as st_pool,
            tc.tile_pool(name="psor", bufs=3, space="PSUM") as or_pool,
        ):
            ones8 = const.tile([P, 2, P], mybir.dt.float8e5)
            nc.vector.memset(ones8[:], 1.0)
            # exp bias -ln(128): keeps exp within fp8e4 range for the fp8
            # row-sum copy; cancels in the softmax normalization
            ebias = const.tile([P, 1], F32)
            nc.vector.memset(ebias[:], -4.852030263919617)

            kvres_cm = tc.tile_pool(name="kvres", bufs=1)
            kvres = kvres_cm.__enter__()
            kT = kvres.tile([P, N_KV, S], BF16)              # [d, kv, t]
            vN = kvres.tile([P, TT, N_KV * HEAD_DIM], BF16)  # [s, s_tile, kv*d]

            xres_cm = tc.tile_pool(name="xres", bufs=1)
            xres = xres_cm.__enter__()
            xt = xres.tile([P, TT, KC, P], BF16)  # [k-part, slice, kc, tj]

            # ---- weight streaming helpers --------------------------------
            def load_head_slab(w_dram, idx):
                """[P, KC, 128] lhsT slab for one q/k head (2 DMAs)."""
                sl = wpool.tile([P, KC, HEAD_DIM], BF16, tag="wst", bufs=3)
                src = w_dram.ap()[idx] if isinstance(idx, int) \
                    else w_dram.ap()[idx[0], idx[1]]
                nc.sync.dma_start(out=sl[:, 0:KH, :], in_=src[:, 0:KH, :])
                nc.sync.dma_start(out=sl[:, KH:KC, :], in_=src[:, KH:KC, :])
                return sl

            # ---- startup DMAs (xt slices interleaved with wv slabs) ------
            def load_sixteenth(w_dram, cc, e):
                sl = wpool.tile([P, 2, NCH], BF16, tag="w16", bufs=17,
                                name="w16t")
                nc.sync.dma_start(
                    out=sl[:],
                    in_=w_dram.ap()[cc, e // 8, :,
                                    (e % 8) * 2:(e % 8) * 2 + 2, :])
                return sl

            def xslice(ts):
                nc.sync.dma_start(out=xt[:, ts, 0:KH, :],
                                  in_=xt_d.ap()[ts, :, 0:KH, :])
                nc.sync.dma_start(out=xt[:, ts, KH:KC, :],
                                  in_=xt_d.ap()[ts, :, KH:KC, :])

            wv_e = [[], []]
            wv_e[0].append(load_sixteenth(wv_d, 0, 0))
            # first slice in quarters so the leading projection matmuls
            # unblock as early as possible
            for kq in range(4):
                nc.sync.dma_start(
                    out=xt[:, 0, kq * 8:(kq + 1) * 8, :],
                    in_=xt_d.ap()[0, :, kq * 8:(kq + 1) * 8, :])
            for e in range(1, 6):
                wv_e[0].append(load_sixteenth(wv_d, 0, e))
            xslice(1)
            for e in range(6, 11):
                wv_e[0].append(load_sixteenth(wv_d, 0, e))
            xslice(2)
            for e in range(11, 16):
                wv_e[0].append(load_sixteenth(wv_d, 0, e))
            xslice(3)
            for e in range(8):
                wv_e[1].append(load_sixteenth(wv_d, 1, e))
            xslice(4)
            xslice(5)
            for e in range(8, 16):
                wv_e[1].append(load_sixteenth(wv_d, 1, e))
            xslice(6)
            xslice(7)
            c2 = const.tile([P, S], BF16)
            nc.sync.dma_start(out=c2[:], in_=c2_d.ap())
            s2n = const.tile([P, S], BF16)
            nc.sync.dma_start(out=s2n[:], in_=s2n_d.ap())

            # ---- compute helpers -----------------------------------------
            def proj_T_mms(slab, tc_, ps, k0, k1):
                """Transposed projection: out[d, t] += slab[kc].T @ xt."""
                for kc in range(k0, k1):
                    nc.tensor.matmul(
                        ps[:],
                        lhsT=slab[:, kc, :],
                        rhs=xt[:, 4 * tc_:4 * tc_ + 4, kc, :],
                        start=(kc == 0),
                        stop=(kc == KC - 1),
                    )

            def rope_t(ps, tc_, dest):
                """RoPE in [d, t] layout: dest = ps*c2 + swap_halves(ps)*s2n."""
                sw = scr_pool.tile([P, NCH], F32, tag="scr")
                nc.vector.tensor_copy(out=sw[0:64, :], in_=ps[64:P, :])
                nc.vector.tensor_copy(out=sw[64:P, :], in_=ps[0:64, :])
                m1 = scr_pool.tile([P, NCH], F32, tag="scr")
                nc.vector.tensor_tensor(m1[:], ps[:],
                                        c2[:, tc_ * NCH:(tc_ + 1) * NCH],
                                        mybir.AluOpType.mult)
                m2 = scr_pool.tile([P, NCH], F32, tag="scr")
                nc.vector.tensor_tensor(m2[:], sw[:],
                                        s2n[:, tc_ * NCH:(tc_ + 1) * NCH],
                                        mybir.AluOpType.mult)
                nc.vector.tensor_tensor(dest, m1[:], m2[:],
                                        mybir.AluOpType.add)

            def make_proj_filler(slab, tc_, dest):
                """Closures: 8x(4 proj matmuls) + rope. First closure
                allocates the psum tile."""
                hold = {}
                clos = []
                for j in range(8):
                    def mm(j=j):
                        if j == 0:
                            hold["ps"] = ps_pool.tile([P, NCH], F32, tag="mm",
                                                      name="pjps")
                        proj_T_mms(slab, tc_, hold["ps"], j * 4, (j + 1) * 4)
                    clos.append(mm)
                def rope():
                    rope_t(hold["ps"], tc_, dest)
                clos.append(rope)
                return clos

            def emit_unit(g, r, qc, qT_use, filler, direct_dest=None):
                """One attention unit (kv-group g, q-head r, q-col chunk qc)
                with PE filler closures woven between score matmuls."""
                pt = pt_pool.tile([P, TT, NCH], BF16, tag="pt")

                def score(st):
                    sps = st_pool.tile([P, NCH], F32, tag="st")
                    nc.tensor.matmul(
                        sps[:],
                        lhsT=kT[:, g, st * P:(st + 1) * P],
                        rhs=qT_use[:, r, qc * NCH:(qc + 1) * NCH],
                        start=True, stop=True,
                    )
                    nc.scalar.activation(
                        pt[:, st, :], sps[:],
                        mybir.ActivationFunctionType.Exp, scale=SCALE,
                        bias=ebias[:],
                    )

                fi = 0
                score(0)
                score(1)
                for st in range(2, TT):
                    if fi < len(filler):
                        filler[fi]()
                        fi += 1
                    score(st)
                # fp8 copy of P^T for the double-row row-sum matmuls
                pt8 = pt8_pool.tile([P, TT, NCH], mybir.dt.float8e5, tag="pt8")
                nc.vector.tensor_copy(out=pt8[:], in_=pt[:])
                while fi < len(filler):
                    filler[fi]()
                    fi += 1

                ops = or_pool.tile([P, NCH], F32, tag="or")
                rps = or_pool.tile([P, NCH], F32, tag="or")
                for st in range(TT):
                    nc.tensor.matmul(
                        ops[:], lhsT=vN[:, st, g * P:(g + 1) * P],
                        rhs=pt[:, st, :], start=(st == 0), stop=(st == TT - 1),
                    )
                for j in range(TT // 2):
                    nc.tensor.matmul(
                        rps[:], lhsT=ones8[:],
                        rhs=pt8[:, 2 * j:2 * j + 2, :],
                        start=(j == 0), stop=(j == TT // 2 - 1),
                        perf_mode=mybir.MatmulPerfMode.DoubleRow,
                    )
                rcb = rcb_pool.tile([P, NCH], F32, tag="rcb")
                nc.vector.reciprocal_approx_fast(out=rcb[:], in_=rps[:])
                if direct_dest is not None:
                    nc.vector.tensor_tensor(direct_dest, ops[:], rcb[:],
                                            mybir.AluOpType.mult)
                else:
                    ot = ot_pool.tile([P, NCH], BF16, tag="ot")
                    nc.vector.tensor_tensor(ot[:], ops[:], rcb[:],
                                            mybir.AluOpType.mult)
                    nc.sync.dma_start(out=otb_d.ap()[qc, :, g * REP + r, :],
                                      in_=ot[:])

            # ---- phase 1: v projection (natural layout) ------------------
            for cc in range(2):
                sixteenths = wv_e[cc]
                for tt in range(TT):
                    ps = ps_pool.tile([P, NCH], F32, tag="mm")
                    for kc in range(KC):
                        nc.tensor.matmul(
                            ps[:],
                            lhsT=xt[:, tt, kc, :],
                            rhs=sixteenths[kc // 2][:, kc % 2, :],
                            start=(kc == 0), stop=(kc == KC - 1),
                        )
                    nc.vector.tensor_copy(
                        out=vN[:, tt, cc * NCH:(cc + 1) * NCH], in_=ps[:])

            # queue q/k head slabs in consumption order (ring-gated)
            wk_slab = [load_head_slab(wk_d, h) for h in range(N_KV)]
            wq_slab = {}
            for h in range(REP):
                wq_slab[(0, h)] = load_head_slab(wq_d, (0, h))
            for h in range(REP):
                wq_slab[(1, h)] = load_head_slab(wq_d, (1, h))

            # ---- phase 2: k projection (transposed) + RoPE ---------------
            for h in range(N_KV):
                for tc_ in range(TC):
                    ps = ps_pool.tile([P, NCH], F32, tag="mm")
                    proj_T_mms(wk_slab[h], tc_, ps, 0, KC)
                    rope_t(ps, tc_, kT[:, h, tc_ * NCH:(tc_ + 1) * NCH])

            # ---- phase 3: q projection for group 0 -----------------------
            qT_cur = qtg_pool.tile([P, REP, S], BF16, tag="qtg")
            for h in range(REP):
                for tc_ in range(TC):
                    ps = ps_pool.tile([P, NCH], F32, tag="mm")
                    proj_T_mms(wq_slab[(0, h)], tc_, ps, 0, KC)
                    rope_t(ps, tc_, qT_cur[:, h, tc_ * NCH:(tc_ + 1) * NCH])

            # ---- groups 0..6: attention woven with next q projection -----
            for g in range(N_KV - 1):
                qT_next = qtg_pool.tile([P, REP, S], BF16, tag="qtg")
                fillers = [
                    make_proj_filler(
                        wq_slab[(g + 1, i // 2)], i % 2,
                        qT_next[:, i // 2, (i % 2) * NCH:(i % 2 + 1) * NCH])
                    for i in range(8)
                ]
                for i in range(8):
                    if g + 2 <= N_KV - 1 and i in (2, 4, 6, 7):
                        hh = {2: 0, 4: 1, 6: 2, 7: 3}[i]
                        wq_slab[(g + 2, hh)] = load_head_slab(wq_d, (g + 2, hh))
                    emit_unit(g, i // 2, i % 2, qT_cur, fillers[i])
                qT_cur = qT_next

            # ---- group 7 + wo --------------------------------------------
            xres_cm.__exit__(None, None, None)
            ores_cm = tc.tile_pool(name="ores", bufs=1)
            ores = ores_cm.__enter__()
            ot_all = ores.tile([P, N_HEADS, S], BF16)  # [d, h, t]

            def bounce(qc):
                for g in range(7):
                    nc.sync.dma_start(
                        out=ot_all[:, g * REP:(g + 1) * REP,
                                   qc * NCH:(qc + 1) * NCH],
                        in_=otb_d.ap()[qc, :, g * REP:(g + 1) * REP, :],
                    )

            def make_wo_chain(slab, ct, tc_):
                """Closures: 8x(4 wo matmuls over heads, slab stationary)
                + copy/DMA tail writing the transposed output."""
                hold = {}
                clos = []
                for j in range(8):
                    def mm(j=j):
                        if j == 0:
                            hold["ps"] = ps_pool.tile([P, NCH], F32, tag="mm",
                                                      name="wops")
                        for h in range(j * 4, (j + 1) * 4):
                            nc.tensor.matmul(
                                hold["ps"][:],
                                lhsT=slab[:, h, :],
                                rhs=ot_all[:, h, tc_ * NCH:(tc_ + 1) * NCH],
                                start=(h == 0), stop=(h == N_HEADS - 1),
                            )
                    clos.append(mm)
                def tail():
                    outt = out_pool.tile([P, NCH], F32, tag="outp")
                    nc.vector.tensor_copy(out=outt[:], in_=hold["ps"][:])
                    nc.sync.dma_start(
                        out=out_d.ap()[ct * P:(ct + 1) * P,
                                       tc_ * NCH:(tc_ + 1) * NCH],
                        in_=outt[:])
                clos.append(tail)
                return clos

            g7dest = lambda r, qc: ot_all[:, 28 + r, qc * NCH:(qc + 1) * NCH]

            bounce(0)
            bounce(1)
            wo_slab = [load_head_slab(wo_d, ct) for ct in range(3)]
            ch0 = [make_wo_chain(wo_slab[ct], ct, 0) for ct in range(3)]
            emit_unit(7, 0, 0, qT_cur, [], direct_dest=g7dest(0, 0))
            emit_unit(7, 1, 0, qT_cur, ch0[0][0:7], direct_dest=g7dest(1, 0))
            emit_unit(7, 2, 0, qT_cur, ch0[1][0:7], direct_dest=g7dest(2, 0))
            emit_unit(7, 3, 0, qT_cur, ch0[2][0:7], direct_dest=g7dest(3, 0))
            for ct in range(3):
                for cl in ch0[ct][7:]:
                    cl()
            ch1 = [make_wo_chain(wo_slab[ct], ct, 1) for ct in range(3)]
            emit_unit(7, 0, 1, qT_cur, ch1[0][0:7], direct_dest=g7dest(0, 1))
            emit_unit(7, 1, 1, qT_cur, ch1[1][0:7], direct_dest=g7dest(1, 1))
            emit_unit(7, 2, 1, qT_cur, ch1[2][0:7], direct_dest=g7dest(2, 1))
            # weave chain(3, tc0) into the last unit: deps (qc0 outputs +
            # bounce(0)) are complete here. Its slab goes through the idle
            # w16 ring and its psum through the or ring - the wst/mm rings'
            # recent slots are freed only by the deferred ch1 tails below,
            # which would deadlock a PE matmul emitted here.
            p3 = []
            for pq in range(4):
                pc = wpool.tile([P, 8, P], BF16, tag="w16", bufs=17, name="p3")
                nc.sync.dma_start(out=pc[:],
                                  in_=wo_d.ap()[3, :, pq * 8:(pq + 1) * 8, :])
                p3.append(pc)

            def make_wo_chain_p(ct, tc_):
                hold = {}
                clos = []
                for j in range(8):
                    def mm(j=j):
                        if j == 0:
                            hold["ps"] = or_pool.tile([P, NCH], F32, tag="or",
                                                      name="wops2")
                        for h in range(j * 4, (j + 1) * 4):
                            nc.tensor.matmul(
                                hold["ps"][:],
                                lhsT=p3[h // 8][:, h % 8, :],
                                rhs=ot_all[:, h, tc_ * NCH:(tc_ + 1) * NCH],
                                start=(h == 0), stop=(h == N_HEADS - 1),
                            )
                    clos.append(mm)
                def tail():
                    outt = out_pool.tile([P, NCH], F32, tag="outp")
                    nc.vector.tensor_copy(out=outt[:], in_=hold["ps"][:])
                    nc.sync.dma_start(
                        out=out_d.ap()[ct * P:(ct + 1) * P,
                                       tc_ * NCH:(tc_ + 1) * NCH],
                        in_=outt[:])
                clos.append(tail)
                return clos

            emit_unit(7, 3, 1, qT_cur, make_wo_chain_p(3, 0),
                      direct_dest=g7dest(3, 1))
            for ct in range(3):
                for cl in ch1[ct][7:]:
                    cl()
            for cl in make_wo_chain_p(3, 1):
                cl()
            for ct in range(4, KC):
                slab = load_head_slab(wo_d, ct)
                for tc_ in range(TC):
                    for cl in make_wo_chain(slab, ct, tc_):
                        cl()
            ores_cm.__exit__(None, None, None)
            kvres_cm.__exit__(None, None, None)

    nc.compile()
    return nc


# host-side input preparation -------------------------------------------------

_ROPE_PERM = np.concatenate([np.arange(0, HEAD_DIM, 2), np.arange(1, HEAD_DIM, 2)])


def _permute_heads(w, n_heads):
    """Permute columns within each head so rotation pairs become
    contiguous (even | odd) halves."""
    w = w.reshape(w.shape[0], n_heads, HEAD_DIM)
    return w[:, :, _ROPE_PERM].reshape(w.shape[0], n_heads * HEAD_DIM)


def _w_layout(w):
    """[DIM, C] f32 -> [C/512, 2, 128, 16, 512] bf16 moving-slab layout."""
    C = w.shape[1]
    wl = w.reshape(2, KH, P, C // NCH, NCH).transpose(3, 0, 2, 1, 4)
    return np.ascontiguousarray(wl).astype(ml_dtypes.bfloat16)


def _prep_shared(cos, sin, wq, wk, wv, wo):
    wq_p = _permute_heads(np.asarray(wq, dtype=np.float32), N_HEADS)
    wk_p = _permute_heads(np.asarray(wk, dtype=np.float32), N_KV)
    # lhsT chunk layouts: [g, h, k-part, kc, d] / [h, k-part, kc, d]
    wq_l = np.ascontiguousarray(
        wq_p.reshape(KC, P, N_KV, REP, HEAD_DIM).transpose(2, 3, 1, 0, 4)
    ).astype(ml_dtypes.bfloat16)
    wk_l = np.ascontiguousarray(
        wk_p.reshape(KC, P, N_KV, HEAD_DIM).transpose(2, 1, 0, 3)
    ).astype(ml_dtypes.bfloat16)
    wv_l = _w_layout(np.asarray(wv, dtype=np.float32))
    # wo lhsT slabs [ct, d, h, c]
    wo_l = np.ascontiguousarray(
        np.asarray(wo, dtype=np.float32)
        .reshape(N_HEADS, HEAD_DIM, KC, P).transpose(2, 1, 0, 3)
    ).astype(ml_dtypes.bfloat16)
    # positions restart at 0 per block, so block 0's tables serve all cores
    c64 = np.asarray(cos[:S], dtype=np.float32).T          # [64, S]
    s64 = np.asarray(sin[:S], dtype=np.float32).T
    c2_l = np.ascontiguousarray(
        np.concatenate([c64, c64], axis=0)).astype(ml_dtypes.bfloat16)
    s2n_l = np.ascontiguousarray(
        np.concatenate([-s64, s64], axis=0)).astype(ml_dtypes.bfloat16)
    return c2_l, s2n_l, wq_l, wk_l, wv_l, wo_l


def _prep_x_block(xb):
    """x block [S, DIM] f32 -> xt [TT, 128, KC, 128] bf16 (transposed,
    slice-major)."""
    xtb = xb.T.reshape(KC, P, TT, P).transpose(2, 1, 0, 3)
    return np.ascontiguousarray(xtb).astype(ml_dtypes.bfloat16)


def kernel(x, cos, sin, wq, wk, wv, wo):
    if "nc" not in _CACHE:
        _CACHE["nc"] = build_nc()
    nc = _CACHE["nc"]

    x = np.asarray(x, dtype=np.float32)
    c2_l, s2n_l, wq_l, wk_l, wv_l, wo_l = _prep_shared(cos, sin, wq, wk, wv, wo)

    in_maps = []
    for b in range(B):
        in_maps.append({
            "xt": _prep_x_block(x[b * S:(b + 1) * S]),
            "c2": c2_l,
            "s2n": s2n_l,
            "wq": wq_l,
            "wk": wk_l,
            "wv": wv_l,
            "wo": wo_l,
        })
    _CACHE["last_in_maps"] = in_maps
    res = run_bass_kernel_spmd(nc, in_maps, core_ids=list(range(B)))
    _CACHE["last_results"] = res
    # per-core output is transposed [DIM, S]
    out = np.concatenate([res.results[b]["out"].T for b in range(B)], axis=0)
    return np.ascontiguousarray(out, dtype=np.float32)
